# revision 2
# baseline (speedup 1.0000x reference)
"""GNN mean-aggregation + 2-layer MLP on 8 Trainium2 NeuronCores.

Reference computation:
    rows = [i;j], cols = [j;i]                      (symmetrized COO)
    agg[n]  = mean over entries (n, c) of conical[c]   (deg clamped to 1)
    out     = relu([radial | agg] @ W1 + b1) @ W2 + b2

Strategy (nodes sharded 8 ways, MLP weights replicated):
  The gather of conical[cols] dominates (3.2M x 64B random reads). Instead of
  per-partition indirect DMA (994ns SWDGE overhead per 128 rows), we use
  InstDMAGatherAnt (dma_gather): thousands of int16 indices per instruction,
  descriptors generated by Q7 core-pairs. queue_num 0-3 selects distinct
  core-pairs, so 4 queues generate descriptors concurrently (~2.4ns/row
  measured vs ~8.1ns/row for indirect DMA).

  Constraints worked around:
   - int16 indices (<32768) + 256B row stride: split neighbors into 4 classes
     by col%4. Class m gathers from table[:, m*16:(m+1)*16] where the table is
     a [25004, 64] f32 view of the packed [100016, 16] conical array; local
     index col//4 then addresses byte col//4*256 + m*64 = row col.
   - elem%256==0 is a transpose-mode-only HW restriction; we build the
     instruction directly with elem_size=16 f32 (64B payloads).
   - single_packet=False so descriptor rings drain while generating
     (single-packet mode overflows the ring above ~1k descriptors).
   - Indices are wrapped [16, n/16] and replicated to all 8 16-partition
     groups (each Q7 core-pair reads its own partition band).

  Host: degree-sort nodes by max per-class count (minimizes padding), groups
  of 8x128 nodes share a uniform per-class pad width K[g,m] so all 8 cores
  run the same program. Device per block of 4 groups: 4 dma_gather (one per
  class/queue) -> per-group per-class strided tensor_reduce + combine + 1/deg
  scale (DVE) -> PE transpose to feature-major -> MLP with stationary weights
  -> node-major DMA out. Host inverse-permutes shards.
"""

import math

import numpy as np

N_CORES = 8
P = 128
GROUP = N_CORES * P  # 1024 nodes per group (one 128-tile per core)
FH = 16  # radial / conical half-width
F = 32
HID = 128
CLS = 4  # col%4 classes
BLOCK_GROUPS = 4  # groups fused per gather/MLP block
MAX_W = 124  # max idx-columns per gather instruction (Q7 scratch limit)


# ---------------------------------------------------------------- host prep


def _host_prep(x, edge_index):
    N = x.shape[0]
    i = edge_index[0].astype(np.int64)
    j = edge_index[1].astype(np.int64)
    rows = np.concatenate([i, j])
    cols = np.concatenate([j, i])
    deg = np.bincount(rows, minlength=N)

    cls = cols % CLS
    cnt = np.zeros((N, CLS), np.int64)
    for m in range(CLS):
        cnt[:, m] = np.bincount(rows[cls == m], minlength=N)

    order = np.argsort(-cnt.max(axis=1), kind="stable")  # new id -> orig id

    # CSR by (row, class)
    key = rows * CLS + cls
    eorder = np.argsort(key, kind="stable")
    loc_sorted = (cols[eorder] // CLS).astype(np.int32)  # local idx in class
    ptr = np.zeros(N * CLS + 1, np.int64)
    ptr[1:] = np.cumsum(np.bincount(key, minlength=N * CLS))

    n_groups = math.ceil(N / GROUP)
    NLOC = math.ceil((N + CLS - 1) // CLS) + 1  # 25001 real rows? use fixed
    NLOC = (N + CLS - 1) // CLS  # 25000 for N=100000
    # pad row: local index NLOC (or beyond) must land in zero rows; table has
    # TROWS = ceil((N + pad)/CLS) rows of 64 f32; we append 8 zero rows of 16f
    PAD_LOC = NLOC  # gathers bytes PAD_LOC*256 + m*64 -> row 4*PAD_LOC+m

    K = np.zeros((n_groups, CLS), np.int64)
    cnt_sorted = cnt[order]
    for g in range(n_groups):
        lo, hi = g * GROUP, min((g + 1) * GROUP, N)
        K[g] = cnt_sorted[lo:hi].max(axis=0)
    S = int(K.sum())

    blocks = []
    g0 = 0
    while g0 < n_groups:
        gs = min(BLOCK_GROUPS, n_groups - g0)
        blocks.append((g0, gs))
        g0 += gs

    total = loc_sorted.shape[0]

    # per-core padded local-index arrays per (group, class): [128, K[g,m]]
    # and the global wrapped idx stream per core.
    idxw_parts = [[] for _ in range(N_CORES)]
    invdeg_all = np.ones((N_CORES, P, n_groups), np.float32)
    radial_all = np.zeros((N_CORES, FH, n_groups * P), np.float32)

    # layout bookkeeping (shared across cores): for each block, for each
    # class, list of (group, col_offset_within_block) and instruction chunks
    block_layout = []  # per block: dict(m -> list of (c0, w)), S_blk, offsets
    for g0, gs in blocks:
        off = 0
        class_info = []  # per class: (col0, [ (group, K, off_in_block) ])
        for m in range(CLS):
            col0 = off
            per_group = []
            for g in range(g0, g0 + gs):
                per_group.append((g, int(K[g, m]), off))
                off += int(K[g, m])
            class_info.append((col0, per_group))
        block_layout.append((class_info, off))  # off == S_blk

    for c in range(N_CORES):
        for (g0, gs), (class_info, S_blk) in zip(blocks, block_layout):
            blkcols = np.empty((P, S_blk), np.int32)
            for m in range(CLS):
                col0, per_group = class_info[m]
                for g, Kg, off in per_group:
                    if Kg == 0:
                        continue
                    lo = g * GROUP + c * P
                    nid = order[lo : lo + P] if lo < N else np.empty(0, np.int64)
                    n_real = max(0, min(N - lo, P))
                    vals = np.full((P, Kg), PAD_LOC, np.int32)
                    if n_real > 0:
                        nid = order[lo : lo + n_real]
                        base = ptr[nid * CLS + m]
                        cg = cnt[nid, m]
                        pos = base[:, None] + np.arange(Kg)[None, :]
                        mask = np.arange(Kg)[None, :] < cg[:, None]
                        vals[:n_real] = np.where(
                            mask, loc_sorted[np.minimum(pos, total - 1)], PAD_LOC
                        )
                    blkcols[:, off : off + Kg] = vals
            # wrap: position i = col*128 + p -> [i%16, i//16]; replicate x8
            flat = blkcols.T.reshape(-1)  # [S_blk*128]
            wrap = flat.reshape(-1, 16).T.astype(np.int16)  # [16, S_blk*8]
            idxw_parts[c].append(np.tile(wrap, (8, 1)))

        r = np.arange(n_groups * P)
        g = r // P
        p = r % P
        newid = g * GROUP + c * P + p
        valid = newid < N
        nid = order[newid[valid]]
        iv = np.ones(n_groups * P, np.float32)
        iv[valid] = (1.0 / np.maximum(deg[nid], 1)).astype(np.float32)
        invdeg_all[c] = iv.reshape(n_groups, P).T
        rad = np.zeros((n_groups * P, FH), np.float32)
        rad[valid] = x[nid, :FH]
        radial_all[c] = rad.T

    idxw_all = [np.ascontiguousarray(np.concatenate(parts, axis=1))
                for parts in idxw_parts]

    # packed conical table: [TROWS, 64] f32 = [4*TROWS, 16]
    TROWS = (N + CLS - 1) // CLS + 2  # includes pad rows (PAD_LOC < TROWS)
    flat_rows = TROWS * CLS
    tbl = np.zeros((flat_rows, FH), np.float32)
    tbl[:N] = x[:, FH:F]
    table = np.ascontiguousarray(tbl.reshape(TROWS, CLS * FH))

    return dict(
        order=order,
        K=K,
        S=S,
        n_groups=n_groups,
        blocks=blocks,
        block_layout=block_layout,
        idxw_all=idxw_all,
        invdeg_all=invdeg_all,
        radial_all=radial_all,
        table=table,
        TROWS=TROWS,
    )


# ------------------------------------------------------------- bass program


def _dma_gather16(nc, out_ap, in_ap, idxs_ap, num_idxs, queue_num):
    """InstDMAGatherAnt, non-transpose: elem_size=16 f32 (64B) at 256B row
    stride. Built directly: bass.dma_gather asserts elem%256==0, which the
    ucode only needs for transpose mode."""
    from concourse import ap_utils, mybir

    eng = nc.gpsimd
    elem_size = in_ap.ap[-1][1]
    elem_step = in_ap.ap[0][0]
    assert ap_utils.ap_is_contiguous(in_ap.ap[1:])
    assert ap_utils.ap_is_contiguous(out_ap.ap[1:])
    assert ap_utils.ap_is_contiguous(idxs_ap.ap[1:])
    assert out_ap.ap[-1][1] == elem_size
    assert out_ap.ap[0][1] * out_ap.ap[1][1] == ((num_idxs + 127) // 128) * 128
    stride_bytes = elem_step * mybir.dt.size(in_ap.dtype)
    stride_bytes_256 = stride_bytes // 256
    assert stride_bytes % 256 == 0 and 0 < stride_bytes_256 < 256
    assert idxs_ap.dtype == mybir.dt.int16

    _in_ap = eng.lower_ap_dma(in_ap, for_custom_bir_dma=True)
    _idxs_ap = eng.lower_ap(idxs_ap)
    _out_ap = eng.lower_ap(out_ap)
    return eng.add_instruction(
        mybir.InstDMAGatherAnt(
            name=eng.bass.get_next_instruction_name(),
            ins=[*_in_ap, _idxs_ap, eng.lower_val_access(eng.to_reg(num_idxs))],
            outs=[_out_ap],
            transpose=False,
            num_idxs=num_idxs,
            elem_size=elem_size,
            stride_bytes_256=stride_bytes_256,
            gen_mode=0,
            single_packet=False,
            queue_num=queue_num,
            sbuf_tokens_per_rank=0,
            sbuf_free_dim_per_rank=0,
            sbuf_free_dim_pad_per_rank=0,
            sbuf_byte_offset=0,
        )
    )


def build_program(K, blocks, block_layout, n_groups, TROWS, S):
    import concourse.tile as tile
    from concourse import bacc, mybir

    f32 = mybir.dt.float32
    i16 = mybir.dt.int16
    AF = mybir.ActivationFunctionType
    ncols = n_groups * P

    nc = bacc.Bacc(None, num_swdge_queues=4)
    table = nc.dram_tensor("table", [TROWS, CLS * FH], f32, kind="ExternalInput")
    idxw = nc.dram_tensor("idxw", [P, 8 * S], i16, kind="ExternalInput")
    radial = nc.dram_tensor("radial", [FH, ncols], f32, kind="ExternalInput")
    invdeg = nc.dram_tensor("invdeg", [P, n_groups], f32, kind="ExternalInput")
    w1a = nc.dram_tensor("w1a", [FH, HID], f32, kind="ExternalInput")
    w1b = nc.dram_tensor("w1b", [FH, HID], f32, kind="ExternalInput")
    w2 = nc.dram_tensor("w2", [HID, F], f32, kind="ExternalInput")
    b1 = nc.dram_tensor("b1", [HID, 1], f32, kind="ExternalInput")
    b2 = nc.dram_tensor("b2", [1, F], f32, kind="ExternalInput")
    out = nc.dram_tensor("out", [ncols, F], f32, kind="ExternalOutput")

    with tile.TileContext(nc) as tc:
        with (
            tc.tile_pool(name="res", bufs=1) as res,
            tc.tile_pool(name="idxp", bufs=3) as idxp,
            tc.tile_pool(name="gat", bufs=3) as gat,
            tc.tile_pool(name="work", bufs=4) as work,
            tc.tile_pool(name="mlp", bufs=2) as mlp,
            tc.tile_pool(name="psum", bufs=2, space="PSUM") as psum,
        ):
            invdeg_sb = res.tile([P, n_groups], f32)
            nc.sync.dma_start(out=invdeg_sb[:], in_=invdeg[:])
            w1a_sb = res.tile([FH, HID], f32)
            nc.sync.dma_start(out=w1a_sb[:], in_=w1a[:])
            w1b_sb = res.tile([FH, HID], f32)
            nc.sync.dma_start(out=w1b_sb[:], in_=w1b[:])
            w2_sb = res.tile([HID, F], f32)
            nc.sync.dma_start(out=w2_sb[:], in_=w2[:])
            b1_sb = res.tile([HID, 1], f32)
            nc.sync.dma_start(out=b1_sb[:], in_=b1[:])
            b2_sb = res.tile([1, F], f32)
            nc.sync.dma_start(out=b2_sb[:], in_=b2[:])
            from concourse.masks import make_identity

            ident_tmp = res.tile([P, P], f32)
            make_identity(nc, ident_tmp[:])
            ident_sb = res.tile([P, P], f32)
            nc.vector.tensor_copy(ident_sb[:], ident_tmp[:])
            ones_sb = res.tile([1, P], f32)
            nc.vector.memset(ones_sb[:], 1.0)

            idx_off = 0  # running column offset into idxw (units of wrapped cols)
            for (g0, gs), (class_info, S_blk) in zip(blocks, block_layout):
                nb = gs * P
                col0 = g0 * P

                idx_sb = idxp.tile([P, 8 * S_blk], i16, tag="idx")
                nc.sync.dma_start(
                    out=idx_sb[:], in_=idxw[:, idx_off : idx_off + 8 * S_blk]
                )

                G = gat.tile([P, S_blk * FH], f32, tag="G")
                for m in range(CLS):
                    col_m0, per_group = class_info[m]
                    wm = sum(kg for _, kg, _ in per_group)
                    c0 = col_m0
                    while wm > 0:
                        w = min(wm, MAX_W)
                        _dma_gather16(
                            nc,
                            G[:, c0 * FH : (c0 + w) * FH].rearrange(
                                "p (c f) -> p c f", f=FH
                            ),
                            table[:, m * FH : (m + 1) * FH],
                            idx_sb[:, 8 * c0 : 8 * (c0 + w)],
                            P * w,
                            queue_num=m,
                        )
                        c0 += w
                        wm -= w

                tr_ps = psum.tile([FH, nb], f32, tag="tr")
                for gi in range(gs):
                    g = g0 + gi
                    tmp = work.tile([P, CLS * FH], f32, tag="tmp")
                    for m in range(CLS):
                        _, per_group = class_info[m]
                        _, Kg, off = per_group[gi]
                        dst = tmp[:, m * FH : (m + 1) * FH]
                        if Kg == 0:
                            nc.vector.memset(dst, 0.0)
                        elif Kg == 1:
                            nc.vector.tensor_copy(
                                dst, G[:, off * FH : (off + 1) * FH]
                            )
                        else:
                            nc.vector.tensor_reduce(
                                out=dst,
                                in_=G[:, off * FH : (off + Kg) * FH].rearrange(
                                    "p (k f) -> p f k", f=FH
                                ),
                                axis=mybir.AxisListType.X,
                                op=mybir.AluOpType.add,
                            )
                    A = work.tile([P, FH], f32, tag="A")
                    nc.vector.tensor_reduce(
                        out=A[:],
                        in_=tmp[:].rearrange("p (m f) -> p f m", f=FH),
                        axis=mybir.AxisListType.X,
                        op=mybir.AluOpType.add,
                    )
                    As = work.tile([P, FH], f32, tag="As")
                    nc.vector.tensor_scalar_mul(
                        As[:], A[:], invdeg_sb[:, g : g + 1]
                    )
                    nc.tensor.transpose(
                        out=tr_ps[:, gi * P : (gi + 1) * P],
                        in_=As[:],
                        identity=ident_sb[:],
                    )

                agg_sb = mlp.tile([FH, nb], f32, tag="agg")
                nc.scalar.activation(agg_sb[:], tr_ps[:], AF.Copy)
                radial_sb = mlp.tile([FH, nb], f32, tag="rad")
                nc.sync.dma_start(
                    out=radial_sb[:], in_=radial[:, col0 : col0 + nb]
                )

                h_ps = psum.tile([HID, nb], f32, tag="h")
                nc.tensor.matmul(
                    h_ps[:], w1a_sb[:], radial_sb[:], start=True, stop=False
                )
                nc.tensor.matmul(h_ps[:], w1b_sb[:], agg_sb[:], start=False, stop=True)
                h_sb = mlp.tile([HID, nb], f32, tag="h_sb")
                nc.scalar.activation(h_sb[:], h_ps[:], AF.Relu, bias=b1_sb[:, :1])

                o_ps = psum.tile([P, gs * F], f32, tag="o")
                for gi in range(gs):
                    nc.tensor.matmul(
                        o_ps[:, gi * F : (gi + 1) * F],
                        ones_sb[:],
                        b2_sb[:],
                        start=True,
                        stop=False,
                    )
                    nc.tensor.matmul(
                        o_ps[:, gi * F : (gi + 1) * F],
                        h_sb[:, gi * P : (gi + 1) * P],
                        w2_sb[:],
                        start=False,
                        stop=True,
                    )
                o_sb = mlp.tile([P, gs * F], f32, tag="o_sb")
                nc.scalar.activation(o_sb[:], o_ps[:], AF.Copy)
                for gi in range(gs):
                    nc.sync.dma_start(
                        out=out[col0 + gi * P : col0 + (gi + 1) * P, :],
                        in_=o_sb[:, gi * F : (gi + 1) * F],
                    )
                idx_off += 8 * S_blk
    return nc


# ------------------------------------------------------------------ driver


def _run(x, edge_index, W1, b1, W2, b2, trace=False):
    from concourse.bass_utils import run_bass_kernel_spmd

    prep = _host_prep(x, edge_index)
    nc = build_program(
        prep["K"],
        prep["blocks"],
        prep["block_layout"],
        prep["n_groups"],
        prep["TROWS"],
        prep["S"],
    )
    if not nc.is_finalized():
        nc.finalize()

    W1 = np.ascontiguousarray(W1, np.float32)
    in_maps = []
    for c in range(N_CORES):
        in_maps.append(
            {
                "table": prep["table"],
                "idxw": prep["idxw_all"][c],
                "radial": np.ascontiguousarray(prep["radial_all"][c]),
                "invdeg": np.ascontiguousarray(prep["invdeg_all"][c]),
                "w1a": np.ascontiguousarray(W1[:FH]),
                "w1b": np.ascontiguousarray(W1[FH:]),
                "w2": np.ascontiguousarray(W2, np.float32),
                "b1": np.ascontiguousarray(b1, np.float32).reshape(HID, 1),
                "b2": np.ascontiguousarray(b2, np.float32).reshape(1, F),
            }
        )
    br = run_bass_kernel_spmd(nc, in_maps, list(range(N_CORES)), trace=trace)

    N = x.shape[0]
    n_groups = prep["n_groups"]
    ncols = n_groups * P
    order = prep["order"]
    result = np.empty((N, F), np.float32)
    r = np.arange(ncols)
    g = r // P
    p = r % P
    for c in range(N_CORES):
        shard = np.asarray(br.results[c]["out"])
        newid = GROUP * g + P * c + p
        valid = newid < N
        result[order[newid[valid]]] = shard[valid]
    return result, br


def kernel(x, edge_index, W1, b1, W2, b2):
    x = np.ascontiguousarray(np.asarray(x), np.float32)
    edge_index = np.ascontiguousarray(np.asarray(edge_index), np.int32)
    result, _ = _run(
        x,
        edge_index,
        np.asarray(W1),
        np.asarray(b1),
        np.asarray(W2),
        np.asarray(b2),
    )
    return result
